# revision 11
# baseline (speedup 1.0000x reference)
"""Trainium2 Bass kernel for nn_DispersiveLoss (B=2048, D=16*768=12288, 8 cores).

Single-launch circulant block decomposition (uniform SPMD):
  x (2048, 12288) -> 16 row-blocks of 128. Core c owns m-blocks {2c, 2c+1}
  and computes two Gram strips G[m, m..m+8 (mod 16)] (width 9 blocks = 1152)
  in fp8 DoubleRow (D on partitions, 48 double-k-chunks, PSUM f32 accum).

  sq_i = ||x_i||^2 for the core's own 256 rows is computed early on the
  otherwise-idle DVE/ACT engines from a natural-layout copy of the rows,
  then u = -(sq - D)/2 (bf16, centered ~N(0,80)) is AllGathered across the
  8 cores on-device and read back rotated via a partition_id-offset dynamic
  DMA. The SAME vector u serves both rank-1 PSUM corrections
  (p = g + u_col + u_row = -(d2 - 2D)/2), folded in with K=1 bf16 matmuls,
  so PSUM holds centered pair values (diag p_ii = D exactly).

  Post-processing per strip is host-weighted-region based (no triangle
  mask): full window W=[0:1152], diag D=[0:128], dist-8 H=[1024:1152] each
  produce (E, S1, S2) partial sums; the host combines with weights
  (1, -1/2, -1/2) in float64 and subtracts the closed-form diagonal
  constants. The strip-1 tail of the k-loop overlaps strip-0's post.
"""

import os

import numpy as np
import ml_dtypes

import concourse.bass as bass
import concourse.mybir as mybir
import concourse.tile as tile
from concourse import bacc
from concourse.bass_utils import run_bass_kernel_spmd

NC_N = 8
B, D = 2048, 12288
BLK = 128
UNION = 1280  # 10 blocks per core in SBUF
STRIPW = 1152  # 9-block strip width
KCH = 96  # k-chunks of 128
KB = 4  # k-chunks per DMA slab
TAU = 0.5
CC = float(2 * D)  # centering constant (E[d2] for N(0,1) rows)
SS = 1.0 / (D * TAU)  # exponent scale
S2E = 2.0 * SS
F32 = mybir.dt.float32
BF16 = mybir.dt.bfloat16
DT_IN = mybir.dt.float8e4
NP_IN = ml_dtypes.float8_e4m3

N_PAIRS = B * (B - 1) // 2

KERNEL_EXEC_NS = []  # filled when KERNEL_TRACE is set (test harness only)

_cache = {}


def _trace_enabled():
    return bool(os.environ.get("KERNEL_TRACE"))


def _build_kernel():
    nc = bacc.Bacc("TRN2", target_bir_lowering=False, debug=False, num_devices=NC_N)
    xT = nc.dram_tensor("xT", [BLK, KCH, UNION], DT_IN, kind="ExternalInput")
    xn = nc.dram_tensor("xn", [2, BLK, D], DT_IN, kind="ExternalInput")
    out_stats = nc.dram_tensor("out_stats", [BLK, 19], F32, kind="ExternalOutput")

    MULT = mybir.AluOpType.mult
    ADD = mybir.AluOpType.add
    EXP = mybir.ActivationFunctionType.Exp
    SQUARE = mybir.ActivationFunctionType.Square
    SQRT = mybir.ActivationFunctionType.Sqrt
    DR = mybir.MatmulPerfMode.DoubleRow
    X = mybir.AxisListType.X

    HD = D // 2  # xn half-block DMA granularity
    segs = [(0, 512), (512, 1024), (1024, 1152)]

    # acc columns: 0 feat; strip s at 1+9s: Ew Ed Eh S1w S1d S1h S2w S2d S2h
    with tile.TileContext(nc) as tc:
        with (
            tc.tile_pool(name="slab", bufs=8) as slab_pool,
            tc.tile_pool(name="xnp", bufs=2) as xnp,
            tc.tile_pool(name="psp", bufs=1, space="PSUM") as psp,
            tc.tile_pool(name="post", bufs=2) as post,
            tc.tile_pool(name="accp", bufs=1) as accp,
            tc.tile_pool(name="dram", bufs=1, space="DRAM") as dram,
        ):
            ps0 = psp.tile([BLK, STRIPW], F32, tag="ps0")
            ps1 = psp.tile([BLK, STRIPW], F32, tag="ps1")
            ps = [ps0, ps1]
            acc = accp.tile([BLK, 19], F32)
            sqa = accp.tile([BLK, 16], F32)
            r = accp.tile([BLK, 2], F32)

            # --- PE pre-warm: trip the HAM busy window while slab 0 DMAs ---
            warm = post.tile([BLK, 512], DT_IN, tag="warm")
            nc.gpsimd.memset(warm[:], 0.0)
            wps = psp.tile([BLK, 512], F32, tag="wps")
            for _ in range(14):
                nc.tensor.matmul(
                    wps[:], warm[:, 0:128], warm[:], start=True, stop=True,
                    skip_group_check=True,
                )
            ones = post.tile([1, 512], BF16, tag="ones")
            nc.gpsimd.memset(ones[:], 1.0)

            KPS = KB // 2  # k-pairs per slab (DoubleRow consumes 2 chunks)
            NSLAB = KCH // KB
            # --- sq of own 256 rows from natural layout (DVE+ACT) ---
            # xn DMAs issue from the vector engine's DMA rings so they run in
            # parallel with the slab stream on the sync rings.
            NSUB = 4
            SUB = HD // NSUB
            for b in range(2):
                for h in range(2):
                    t = xnp.tile([BLK, HD], DT_IN, tag="xin")
                    nc.scalar.dma_start(t[:], xn[b, :, h * HD : (h + 1) * HD])
                    for j in range(NSUB):
                        col_i = (b * 2 + h) * NSUB + j
                        col = sqa[:, col_i : col_i + 1]
                        ts_ = t[:, j * SUB : (j + 1) * SUB]
                        scr = xnp.tile([BLK, SUB], F32, tag="xsc")
                        if b == 0:
                            nc.vector.scalar_tensor_tensor(
                                out=scr[:], in0=ts_, scalar=1.0, in1=ts_,
                                op0=MULT, op1=MULT, accum_out=col,
                            )
                        else:
                            nc.scalar.activation(
                                scr[:], ts_, SQUARE, accum_out=col,
                            )
            nc.vector.tensor_reduce(r[:, 0:1], sqa[:, 0:8], X, ADD)
            nc.vector.tensor_reduce(r[:, 1:2], sqa[:, 8:16], X, ADD)
            # feat partial: sum sqrt(sq) (ACT Sqrt table, then preload Exp so
            # the table is resident for the post phase)
            fscr = accp.tile([BLK, 2], F32)
            nc.scalar.activation(fscr[:], r[:], SQRT, accum_out=acc[:, 0:1])
            zcol = accp.tile([BLK, 1], F32)
            nc.gpsimd.memset(zcol[:], 0.0)
            zscr = accp.tile([BLK, 1], F32)
            nc.scalar.activation(zscr[:], zcol[:], EXP)
            # u = -(sq - D)/2 in bf16; AllGather across cores
            uown = accp.tile([BLK, 2], BF16)
            nc.vector.tensor_scalar(
                out=uown[:], in0=r[:], scalar1=-float(D), scalar2=-0.5,
                op0=ADD, op1=MULT,
            )
            ub = dram.tile([256], BF16)
            ug = dram.tile([4096], BF16)
            nc.gpsimd.dma_start(ub[:].rearrange("(b p) -> p b", b=2), uown[:])
            nc.gpsimd.collective_compute(
                "AllGather",
                mybir.AluOpType.bypass,
                replica_groups=[list(range(NC_N))],
                ins=[ub[:]],
                outs=[ug[0:2048]],
            )
            nc.gpsimd.dma_start(ug[2048:4096], ug[0:2048])
            pid = nc.gpsimd.partition_id()
            urow = accp.tile([1, UNION], BF16)
            nc.gpsimd.dma_start(
                urow[:],
                ug[bass.DynSlice(pid * 256, UNION)].rearrange(
                    "(a b) -> a b", a=1
                ),
            )

            sts = []
            for kb in range(NSLAB):
                st = slab_pool.tile([BLK, KB, UNION], DT_IN, tag="slab")
                sts.append(st)
                nc.sync.dma_start(st[:], xT[:, kb * KB : (kb + 1) * KB, :])

                if kb < NSLAB - 4:
                    for kp in range(KPS):
                        ii = 2 * kp
                        for s in range(2):
                            off = 128 * s
                            lhs = st[:, ii : ii + 2, off : off + 128]
                            for c0, c1 in segs:
                                nc.tensor.matmul(
                                    ps[s][:, c0:c1],
                                    lhs,
                                    st[:, ii : ii + 2, off + c0 : off + c1],
                                    start=(kb == 0 and kp == 0),
                                    stop=False,
                                    perf_mode=DR,
                                )

            def strip_tail_mms(s):
                off = 128 * s
                for kb in range(NSLAB - 4, NSLAB):
                    st = sts[kb]
                    for kp in range(KPS):
                        ii = 2 * kp
                        lhs = st[:, ii : ii + 2, off : off + 128]
                        for c0, c1 in segs:
                            nc.tensor.matmul(
                                ps[s][:, c0:c1],
                                lhs,
                                st[:, ii : ii + 2, off + c0 : off + c1],
                                start=False,
                                stop=False,
                                perf_mode=DR,
                            )
                # rank-1 corrections: p += 1^T @ u_col  and  u_row^T @ 1
                for c0, c1 in segs:
                    nc.tensor.matmul(
                        ps[s][:, c0:c1],
                        ones[:, 0:128],
                        urow[:, off + c0 : off + c1],
                        start=False,
                        stop=False,
                    )
                for j, (c0, c1) in enumerate(segs):
                    nc.tensor.matmul(
                        ps[s][:, c0:c1],
                        urow[:, off : off + 128],
                        ones[:, 0 : c1 - c0],
                        start=False,
                        stop=(j == len(segs) - 1),
                    )

            def strip_post(s):
                p = ps[s]
                base = 1 + 9 * s
                scr = post.tile([BLK, STRIPW], F32, tag="scr")
                nc.scalar.activation(
                    scr[:], p[:, 0:STRIPW], EXP, scale=S2E,
                    accum_out=acc[:, base : base + 1],
                )
                pS = post.tile([BLK, STRIPW], F32, tag="pS")
                nc.vector.tensor_scalar(
                    out=pS[:], in0=p[:, 0:STRIPW], scalar1=1.0, scalar2=0.0,
                    op0=MULT, op1=ADD, accum_out=acc[:, base + 3 : base + 4],
                )
                scrd = post.tile([BLK, STRIPW], F32, tag="scrd")
                nc.vector.scalar_tensor_tensor(
                    out=scrd[:], in0=pS[:], scalar=1.0, in1=p[:, 0:STRIPW],
                    op0=MULT, op1=MULT, accum_out=acc[:, base + 6 : base + 7],
                )
                nc.vector.tensor_reduce(
                    acc[:, base + 1 : base + 2], scr[:, 0:128], X, ADD
                )
                nc.vector.tensor_reduce(
                    acc[:, base + 2 : base + 3], scr[:, 1024:1152], X, ADD
                )
                nc.vector.tensor_reduce(
                    acc[:, base + 4 : base + 5], pS[:, 0:128], X, ADD
                )
                nc.vector.tensor_reduce(
                    acc[:, base + 5 : base + 6], pS[:, 1024:1152], X, ADD
                )
                s2d = post.tile([BLK, BLK], F32, tag="s2d")
                nc.vector.scalar_tensor_tensor(
                    out=s2d[:], in0=pS[:, 0:128], scalar=1.0, in1=pS[:, 0:128],
                    op0=MULT, op1=MULT, accum_out=acc[:, base + 7 : base + 8],
                )
                s2h = post.tile([BLK, BLK], F32, tag="s2h")
                nc.vector.scalar_tensor_tensor(
                    out=s2h[:], in0=pS[:, 1024:1152], scalar=1.0,
                    in1=pS[:, 1024:1152],
                    op0=MULT, op1=MULT, accum_out=acc[:, base + 8 : base + 9],
                )

            strip_tail_mms(0)
            strip_post(0)
            strip_tail_mms(1)
            strip_post(1)

            nc.sync.dma_start(out_stats[:], acc[:])
    nc.compile()
    return nc


def _get(name, builder):
    if name not in _cache:
        _cache[name] = builder()
    return _cache[name]


def _run(nc, in_maps, tag):
    if _trace_enabled():
        try:
            import profhook

            profhook.install()
        except Exception:
            pass
        import tempfile

        res = run_bass_kernel_spmd(
            nc, in_maps, list(range(NC_N)), trace=True,
            tmpdir=tempfile.mkdtemp(prefix=f"ktrace_{tag}_"),
        )
        KERNEL_EXEC_NS.append((tag, res.exec_time_ns))
        return res.results
    return run_bass_kernel_spmd(nc, in_maps, list(range(NC_N))).results


def kernel(features):
    x = np.asarray(features).reshape(B, D)
    xbf = x.astype(NP_IN)

    xT_full = np.ascontiguousarray(xbf.T)  # (D, B)
    in_maps = []
    for c in range(NC_N):
        cols = (256 * c + np.arange(UNION)) % B
        xu = xT_full[:, cols].reshape(KCH, BLK, UNION).transpose(1, 0, 2)
        in_maps.append(
            {
                "xT": np.ascontiguousarray(xu),
                "xn": np.ascontiguousarray(
                    xbf[256 * c : 256 * c + 256]
                ).reshape(2, BLK, D),
            }
        )
    nc_k = _get("main", _build_kernel)
    res = _run(nc_k, in_maps, "main")

    # ---- host combine: weighted regions in float64 ----
    FT = E = T1 = T2 = 0.0
    for c in range(NC_N):
        o = res[c]["out_stats"].astype(np.float64)
        FT += o[:, 0].sum()
        for s in range(2):
            b = 1 + 9 * s
            E += o[:, b].sum() - 0.5 * (o[:, b + 1].sum() + o[:, b + 2].sum())
            T1 += o[:, b + 3].sum() - 0.5 * (o[:, b + 4].sum() + o[:, b + 5].sum())
            T2 += o[:, b + 6].sum() - 0.5 * (o[:, b + 7].sum() + o[:, b + 8].sum())
    # closed-form diagonal constants: p_ii = D exactly
    E -= 1024.0 * np.exp(S2E * D)
    T1 -= 1024.0 * D
    T2 -= 1024.0 * D * D

    N = float(N_PAIRS)
    mean_u = -2.0 * T1 / N
    mean = (mean_u + CC) / D
    var_u = (4.0 * T2 - N * mean_u * mean_u) / (N - 1.0)
    std = np.sqrt(var_u) / D
    loss = CC * SS - np.log(E) + np.log(N)
    feat_norm = FT / B

    return (
        np.float32(loss),
        np.float32(feat_norm),
        np.float32(mean),
        np.float32(std),
    )


if __name__ == "__main__":
    f = np.random.default_rng(0).standard_normal((B, 16, 768), dtype=np.float32)
    print(kernel(features=f))


# revision 12
# speedup vs baseline: 1.0526x; 1.0526x over previous
"""Trainium2 Bass kernel for nn_DispersiveLoss (B=2048, D=16*768=12288, 8 cores).

Single-launch circulant block decomposition (uniform SPMD):
  x (2048, 12288) -> 16 row-blocks of 128. Core c owns m-blocks {2c, 2c+1}
  and computes two Gram strips G[m, m..m+8 (mod 16)] (width 9 blocks = 1152)
  in fp8 DoubleRow (D on partitions, 48 double-k-chunks, PSUM f32 accum).

  sq_i = ||x_i||^2 for the core's own 256 rows is computed early on the
  otherwise-idle DVE/ACT engines from a natural-layout copy of the rows,
  then u = -(sq - D)/2 (bf16, centered ~N(0,80)) is AllGathered across the
  8 cores on-device and read back rotated via a partition_id-offset dynamic
  DMA. The SAME vector u serves both rank-1 PSUM corrections
  (p = g + u_col + u_row = -(d2 - 2D)/2), folded in with K=1 bf16 matmuls,
  so PSUM holds centered pair values (diag p_ii = D exactly).

  Post-processing per strip is host-weighted-region based (no triangle
  mask): full window W=[0:1152], diag D=[0:128], dist-8 H=[1024:1152] each
  produce (E, S1, S2) partial sums; the host combines with weights
  (1, -1/2, -1/2) in float64 and subtracts the closed-form diagonal
  constants. The strip-1 tail of the k-loop overlaps strip-0's post.
"""

import os

import numpy as np
import ml_dtypes

import concourse.bass as bass
import concourse.mybir as mybir
import concourse.tile as tile
from concourse import bacc
from concourse.bass_utils import run_bass_kernel_spmd

NC_N = 8
B, D = 2048, 12288
BLK = 128
UNION = 1280  # 10 blocks per core in SBUF
STRIPW = 1152  # 9-block strip width
KCH = 96  # k-chunks of 128
KB = 4  # k-chunks per DMA slab
TAU = 0.5
CC = float(2 * D)  # centering constant (E[d2] for N(0,1) rows)
SS = 1.0 / (D * TAU)  # exponent scale
S2E = 2.0 * SS
F32 = mybir.dt.float32
BF16 = mybir.dt.bfloat16
DT_IN = mybir.dt.float8e4
NP_IN = ml_dtypes.float8_e4m3

N_PAIRS = B * (B - 1) // 2

KERNEL_EXEC_NS = []  # filled when KERNEL_TRACE is set (test harness only)

_cache = {}


def _trace_enabled():
    return bool(os.environ.get("KERNEL_TRACE"))


def _build_kernel():
    nc = bacc.Bacc("TRN2", target_bir_lowering=False, debug=False, num_devices=NC_N)
    xT = nc.dram_tensor("xT", [BLK, KCH, UNION], DT_IN, kind="ExternalInput")
    xn = nc.dram_tensor("xn", [2, BLK, D], DT_IN, kind="ExternalInput")
    out_stats = nc.dram_tensor("out_stats", [BLK, 19], F32, kind="ExternalOutput")

    MULT = mybir.AluOpType.mult
    ADD = mybir.AluOpType.add
    EXP = mybir.ActivationFunctionType.Exp
    SQUARE = mybir.ActivationFunctionType.Square
    SQRT = mybir.ActivationFunctionType.Sqrt
    DR = mybir.MatmulPerfMode.DoubleRow
    X = mybir.AxisListType.X

    HD = D // 2  # xn half-block DMA granularity
    segs = [(0, 512), (512, 1024), (1024, 1152)]

    # acc columns: 0 feat; strip s at 1+9s: Ew Ed Eh S1w S1d S1h S2w S2d S2h
    with tile.TileContext(nc) as tc:
        with (
            tc.tile_pool(name="slab", bufs=8) as slab_pool,
            tc.tile_pool(name="xnp", bufs=2) as xnp,
            tc.tile_pool(name="psp", bufs=1, space="PSUM") as psp,
            tc.tile_pool(name="post", bufs=2) as post,
            tc.tile_pool(name="accp", bufs=1) as accp,
            tc.tile_pool(name="dram", bufs=1, space="DRAM") as dram,
        ):
            ps0 = psp.tile([BLK, STRIPW], F32, tag="ps0")
            ps1 = psp.tile([BLK, STRIPW], F32, tag="ps1")
            ps = [ps0, ps1]
            acc = accp.tile([BLK, 19], F32)
            sqa = accp.tile([BLK, 16], F32)
            r = accp.tile([BLK, 2], F32)

            # --- PE pre-warm: trip the HAM busy window while slab 0 DMAs ---
            warm = post.tile([BLK, 512], DT_IN, tag="warm")
            nc.gpsimd.memset(warm[:], 0.0)
            wps = psp.tile([BLK, 512], F32, tag="wps")
            for _ in range(14):
                nc.tensor.matmul(
                    wps[:], warm[:, 0:128], warm[:], start=True, stop=True,
                    skip_group_check=True,
                )
            ones = post.tile([1, 512], BF16, tag="ones")
            nc.gpsimd.memset(ones[:], 1.0)

            KPS = KB // 2  # k-pairs per slab (DoubleRow consumes 2 chunks)
            NSLAB = KCH // KB
            # --- sq of own 256 rows from natural layout (DVE+ACT) ---
            # xn chunks go at the HEAD of the sync DMA stream so every core's
            # sq -> AllGather trigger fires by ~13us; the collective (a
            # barrier across cores, ~40us) then completes well before the
            # k-loop ends and never blocks the PE tail.
            NSUB = 4
            SUB = HD // NSUB
            for b in range(2):
                for h in range(2):
                    t = xnp.tile([BLK, HD], DT_IN, tag="xin")
                    nc.sync.dma_start(t[:], xn[b, :, h * HD : (h + 1) * HD])
                    for j in range(NSUB):
                        col_i = (b * 2 + h) * NSUB + j
                        col = sqa[:, col_i : col_i + 1]
                        ts_ = t[:, j * SUB : (j + 1) * SUB]
                        scr = xnp.tile([BLK, SUB], F32, tag="xsc")
                        if b == 0:
                            nc.vector.scalar_tensor_tensor(
                                out=scr[:], in0=ts_, scalar=1.0, in1=ts_,
                                op0=MULT, op1=MULT, accum_out=col,
                            )
                        else:
                            nc.scalar.activation(
                                scr[:], ts_, SQUARE, accum_out=col,
                            )
            nc.vector.tensor_reduce(r[:, 0:1], sqa[:, 0:8], X, ADD)
            nc.vector.tensor_reduce(r[:, 1:2], sqa[:, 8:16], X, ADD)
            # feat partial: sum sqrt(sq) (ACT Sqrt table, then preload Exp so
            # the table is resident for the post phase)
            fscr = accp.tile([BLK, 2], F32)
            nc.scalar.activation(fscr[:], r[:], SQRT, accum_out=acc[:, 0:1])
            zcol = accp.tile([BLK, 1], F32)
            nc.gpsimd.memset(zcol[:], 0.0)
            zscr = accp.tile([BLK, 1], F32)
            nc.scalar.activation(zscr[:], zcol[:], EXP)
            # u = -(sq - D)/2 in bf16; AllGather across cores
            uown = accp.tile([BLK, 2], BF16)
            nc.vector.tensor_scalar(
                out=uown[:], in0=r[:], scalar1=-float(D), scalar2=-0.5,
                op0=ADD, op1=MULT,
            )
            ub = dram.tile([256], BF16)
            ug = dram.tile([4096], BF16)
            nc.gpsimd.dma_start(ub[:].rearrange("(b p) -> p b", b=2), uown[:])
            nc.gpsimd.collective_compute(
                "AllGather",
                mybir.AluOpType.bypass,
                replica_groups=[list(range(NC_N))],
                ins=[ub[:]],
                outs=[ug[0:2048]],
            )
            nc.gpsimd.dma_start(ug[2048:4096], ug[0:2048])
            pid = nc.gpsimd.partition_id()
            urow = accp.tile([1, UNION], BF16)
            nc.gpsimd.dma_start(
                urow[:],
                ug[bass.DynSlice(pid * 256, UNION)].rearrange(
                    "(a b) -> a b", a=1
                ),
            )

            sts = []
            for kb in range(NSLAB):
                st = slab_pool.tile([BLK, KB, UNION], DT_IN, tag="slab")
                sts.append(st)
                nc.sync.dma_start(st[:], xT[:, kb * KB : (kb + 1) * KB, :])

                if kb < NSLAB - 4:
                    for kp in range(KPS):
                        ii = 2 * kp
                        for s in range(2):
                            off = 128 * s
                            lhs = st[:, ii : ii + 2, off : off + 128]
                            for c0, c1 in segs:
                                nc.tensor.matmul(
                                    ps[s][:, c0:c1],
                                    lhs,
                                    st[:, ii : ii + 2, off + c0 : off + c1],
                                    start=(kb == 0 and kp == 0),
                                    stop=False,
                                    perf_mode=DR,
                                )

            def strip_tail_mms(s):
                off = 128 * s
                for kb in range(NSLAB - 4, NSLAB):
                    st = sts[kb]
                    for kp in range(KPS):
                        ii = 2 * kp
                        lhs = st[:, ii : ii + 2, off : off + 128]
                        for c0, c1 in segs:
                            nc.tensor.matmul(
                                ps[s][:, c0:c1],
                                lhs,
                                st[:, ii : ii + 2, off + c0 : off + c1],
                                start=False,
                                stop=False,
                                perf_mode=DR,
                            )
                # rank-1 corrections: p += 1^T @ u_col  and  u_row^T @ 1
                for c0, c1 in segs:
                    nc.tensor.matmul(
                        ps[s][:, c0:c1],
                        ones[:, 0:128],
                        urow[:, off + c0 : off + c1],
                        start=False,
                        stop=False,
                    )
                for j, (c0, c1) in enumerate(segs):
                    nc.tensor.matmul(
                        ps[s][:, c0:c1],
                        urow[:, off : off + 128],
                        ones[:, 0 : c1 - c0],
                        start=False,
                        stop=(j == len(segs) - 1),
                    )

            def strip_post(s):
                p = ps[s]
                base = 1 + 9 * s
                scr = post.tile([BLK, STRIPW], F32, tag="scr")
                nc.scalar.activation(
                    scr[:], p[:, 0:STRIPW], EXP, scale=S2E,
                    accum_out=acc[:, base : base + 1],
                )
                pS = post.tile([BLK, STRIPW], F32, tag="pS")
                nc.vector.tensor_scalar(
                    out=pS[:], in0=p[:, 0:STRIPW], scalar1=1.0, scalar2=0.0,
                    op0=MULT, op1=ADD, accum_out=acc[:, base + 3 : base + 4],
                )
                scrd = post.tile([BLK, STRIPW], F32, tag="scrd")
                nc.vector.scalar_tensor_tensor(
                    out=scrd[:], in0=pS[:], scalar=1.0, in1=p[:, 0:STRIPW],
                    op0=MULT, op1=MULT, accum_out=acc[:, base + 6 : base + 7],
                )
                nc.vector.tensor_reduce(
                    acc[:, base + 1 : base + 2], scr[:, 0:128], X, ADD
                )
                nc.vector.tensor_reduce(
                    acc[:, base + 2 : base + 3], scr[:, 1024:1152], X, ADD
                )
                nc.vector.tensor_reduce(
                    acc[:, base + 4 : base + 5], pS[:, 0:128], X, ADD
                )
                nc.vector.tensor_reduce(
                    acc[:, base + 5 : base + 6], pS[:, 1024:1152], X, ADD
                )
                s2d = post.tile([BLK, BLK], F32, tag="s2d")
                nc.vector.scalar_tensor_tensor(
                    out=s2d[:], in0=pS[:, 0:128], scalar=1.0, in1=pS[:, 0:128],
                    op0=MULT, op1=MULT, accum_out=acc[:, base + 7 : base + 8],
                )
                s2h = post.tile([BLK, BLK], F32, tag="s2h")
                nc.vector.scalar_tensor_tensor(
                    out=s2h[:], in0=pS[:, 1024:1152], scalar=1.0,
                    in1=pS[:, 1024:1152],
                    op0=MULT, op1=MULT, accum_out=acc[:, base + 8 : base + 9],
                )

            strip_tail_mms(0)
            strip_post(0)
            strip_tail_mms(1)
            strip_post(1)

            nc.sync.dma_start(out_stats[:], acc[:])
    nc.compile()
    return nc


def _get(name, builder):
    if name not in _cache:
        _cache[name] = builder()
    return _cache[name]


def _run(nc, in_maps, tag):
    if _trace_enabled():
        try:
            import profhook

            profhook.install()
        except Exception:
            pass
        import tempfile

        res = run_bass_kernel_spmd(
            nc, in_maps, list(range(NC_N)), trace=True,
            tmpdir=tempfile.mkdtemp(prefix=f"ktrace_{tag}_"),
        )
        KERNEL_EXEC_NS.append((tag, res.exec_time_ns))
        return res.results
    return run_bass_kernel_spmd(nc, in_maps, list(range(NC_N))).results


def kernel(features):
    x = np.asarray(features).reshape(B, D)
    xbf = x.astype(NP_IN)

    xT_full = np.ascontiguousarray(xbf.T)  # (D, B)
    in_maps = []
    for c in range(NC_N):
        cols = (256 * c + np.arange(UNION)) % B
        xu = xT_full[:, cols].reshape(KCH, BLK, UNION).transpose(1, 0, 2)
        in_maps.append(
            {
                "xT": np.ascontiguousarray(xu),
                "xn": np.ascontiguousarray(
                    xbf[256 * c : 256 * c + 256]
                ).reshape(2, BLK, D),
            }
        )
    nc_k = _get("main", _build_kernel)
    res = _run(nc_k, in_maps, "main")

    # ---- host combine: weighted regions in float64 ----
    FT = E = T1 = T2 = 0.0
    for c in range(NC_N):
        o = res[c]["out_stats"].astype(np.float64)
        FT += o[:, 0].sum()
        for s in range(2):
            b = 1 + 9 * s
            E += o[:, b].sum() - 0.5 * (o[:, b + 1].sum() + o[:, b + 2].sum())
            T1 += o[:, b + 3].sum() - 0.5 * (o[:, b + 4].sum() + o[:, b + 5].sum())
            T2 += o[:, b + 6].sum() - 0.5 * (o[:, b + 7].sum() + o[:, b + 8].sum())
    # closed-form diagonal constants: p_ii = D exactly
    E -= 1024.0 * np.exp(S2E * D)
    T1 -= 1024.0 * D
    T2 -= 1024.0 * D * D

    N = float(N_PAIRS)
    mean_u = -2.0 * T1 / N
    mean = (mean_u + CC) / D
    var_u = (4.0 * T2 - N * mean_u * mean_u) / (N - 1.0)
    std = np.sqrt(var_u) / D
    loss = CC * SS - np.log(E) + np.log(N)
    feat_norm = FT / B

    return (
        np.float32(loss),
        np.float32(feat_norm),
        np.float32(mean),
        np.float32(std),
    )


if __name__ == "__main__":
    f = np.random.default_rng(0).standard_normal((B, 16, 768), dtype=np.float32)
    print(kernel(features=f))
